# revision 50
# baseline (speedup 1.0000x reference)
"""Multi-head attention (ALiBi, symmetric) Trainium2 kernel.

Problem: B=2, L=2048, C=1024, H=16 heads, D=64 (torch-Linear projections,
symmetric ALiBi bias, softmax, output projection + bias).

Sharding: 8 cores = batch (2) x head-group (4). Head groups take one head
from each "slope tier" so banded-attention work is balanced:
    g0=[15,9,7,3], g1=[14,8,6,2], g2=[13,10,5,1], g3=[12,11,4,0]
Each core computes its 4 heads' attention and a partial output projection
(columns of Wo restricted to its heads); the host sums the 4 partials per
batch and adds bo. No on-device collectives.

Key tricks:
  - softmax without max-subtraction (scores bounded: qk/8 ~ +-3, bias <= 0)
  - ALiBi bias applied INSIDE the QK^T matmul via augmentation rows:
    bias = +-(s*j - s*i) encoded with 11 exact bf16 bit-planes of j (and of
    i), ones rows on the opposite operand. Two kh variants (lower/upper
    triangle); diagonal 128x128 chunks get their bias via an extra PE
    matmul (tdiag^T through an identity rhs, accumulated in PSUM) so no
    vector-engine op sits in the QK->exp chain.
  - S^T layout ([keys, queries]) so the softmax denominator comes free from
    a ones-column appended to V in the PV matmul, and attn^T feeds both the
    PV and output-projection matmuls with no transposes.
  - per-query normalization folded into the attn-output copy (multiply by
    DMA-broadcast reciprocal of the denominator rows, batched per pair).
  - banded attention: tiles with min-distance bias < -7 are skipped
    (exp < 1e-3 relative).

Scheduling (the PE clock ramps 0.65->1.2->2.4 GHz with sustained use, so
the tensor-engine queue is kept dense):
  - input DMAs are chunked and ordered so the first projection matmul
    starts ~4us in, with the rest of the loads streaming underneath.
  - PSUM budget: scores 2x2-bank, PV accumulators 4x1-bank, and a separate
    2x1-bank pool for V-proj/o-proj so o-proj(m) overlaps attention(m+1).
  - attention of pair 1 slot 0 is emitted between norm(0) and oproj(0) so
    the PE never waits on the normalization DMA round-trip.

All matmuls bf16 inputs / fp32 PSUM accumulation; output partials bf16.
"""

import os
import numpy as np
import ml_dtypes

import concourse.bass as bass
import concourse.tile as tile
from concourse import bacc, mybir

H = 16
D = 64
L = 2048
C = 1024
NB = 11            # bit planes for j/i (2048 = 2^11)
BAND_T = 7.0       # drop tiles with slope*dist > BAND_T
JT = 128           # key tile (partitions)
IT = 512           # query tile granularity
PAIR = 2 * IT      # 1024-wide query pair
N_CORES = 8
GROUPS = [[15, 9, 7, 3], [14, 8, 6, 2], [13, 10, 5, 1], [12, 11, 4, 0]]
F32 = mybir.dt.float32
BF16 = mybir.dt.bfloat16
BF16_NP = ml_dtypes.bfloat16

_last_results = None


def _slopes():
    start = 2.0 ** (-(2.0 ** -(np.log2(H) - 3)))
    return np.array([start * (start ** i) for i in range(H)], dtype=np.float32)


SLOPES = _slopes()
# slot s of group g holds head GROUPS[g][s]; every core keeps the union band
# of its slot tier, whose widest member is the tier's smallest slope.
TIER_SLOPE = [SLOPES[15], SLOPES[11], SLOPES[7], SLOPES[3]]


def _win(s, Jt, It_):
    """Kept 128-query chunks (contiguous window) of query tile It_ vs key
    tile Jt, at 128-granularity; None if empty."""
    J0, I0 = Jt * JT, It_ * IT
    cs = []
    for c in range(IT // JT):
        Ic = I0 + c * JT
        dist = max(0, max(J0 - (Ic + JT - 1), Ic - (J0 + JT - 1)))
        if TIER_SLOPE[s] * dist <= BAND_T:
            cs.append(c)
    if not cs:
        return None
    assert cs == list(range(cs[0], cs[-1] + 1))
    return (cs[0], cs[-1] + 1)


def build_bass():
    nc = bacc.Bacc("TRN2", target_bir_lowering=False, debug=False,
                   num_devices=N_CORES)

    KT_ = C // 128
    xT_d = nc.dram_tensor("xT", [128, KT_, L], BF16, kind="ExternalInput")
    wqT_d = nc.dram_tensor("wqT", [128, KT_, 4 * D], BF16,
                           kind="ExternalInput")
    wkT_d = nc.dram_tensor("wkT", [128, KT_, 4 * D], BF16,
                           kind="ExternalInput")
    wvT_d = nc.dram_tensor("wvT", [128, KT_, 4 * D], BF16,
                           kind="ExternalInput")
    woT_d = nc.dram_tensor("woT", [2, 128, C], BF16, kind="ExternalInput")
    augq_d = nc.dram_tensor("augq", [4, 2 * NB, L], BF16, kind="ExternalInput")
    augkl_d = nc.dram_tensor("augkl", [4, 2 * NB, L], BF16, kind="ExternalInput")
    augku_d = nc.dram_tensor("augku", [4, 2 * NB, L], BF16, kind="ExternalInput")
    tdiag_d = nc.dram_tensor("tdiag", [4, JT, JT], BF16, kind="ExternalInput")
    ident_d = nc.dram_tensor("ident", [JT, JT], BF16, kind="ExternalInput")
    out_d = nc.dram_tensor("out", [L, C], BF16, kind="ExternalOutput")

    KT = C // 128      # 8 contraction tiles
    AUGP = D + 2 * NB  # 86 partitions for augmented qk matmuls
    NCB = L // IT      # 4 column blocks of xT

    with tile.TileContext(nc) as tc:
        with (
            tc.tile_pool(name="const", bufs=1) as const,
            tc.tile_pool(name="psm", bufs=2, space="PSUM") as psm,
            tc.tile_pool(name="pspv", bufs=2, space="PSUM") as pspv,
            tc.tile_pool(name="psy", bufs=2, space="PSUM") as psy,
            tc.tile_pool(name="ppool", bufs=8) as ppool,
            tc.tile_pool(name="ypool", bufs=20) as ypool,
            tc.tile_pool(name="rpool", bufs=2) as rpool,
            tc.tile_pool(name="dnb", bufs=3) as dnb,
            tc.tile_pool(name="recpool", bufs=8) as recpool,
            tc.tile_pool(name="upool", bufs=8) as upool,
            tc.tile_pool(name="dpool", bufs=4, space="DRAM") as dpool,
        ):
            # ---- warm the exp activation table while DMAs stream ----
            scr = const.tile([1, 16], F32, tag="scr", name="scr")
            scr2 = const.tile([1, 16], F32, tag="scr2", name="scr2")
            nc.vector.memset(scr, 0.0)
            nc.scalar.activation(out=scr2, in_=scr,
                                 func=mybir.ActivationFunctionType.Exp)

            # ---- load inputs (host pre-arranged [128, KT, *] so each is
            # one DMA; ordered so q-proj can start ~4us in) ----
            xT_sb = const.tile([128, KT, L], BF16)
            wqT_sb = const.tile([128, KT, 4 * D], BF16)
            wkT_sb = const.tile([128, KT, 4 * D], BF16)
            wvT_sb = const.tile([128, KT, 4 * D], BF16)

            def load_x_cb(cb):
                csl = slice(cb * IT, (cb + 1) * IT)
                nc.sync.dma_start(out=xT_sb[:, :, csl],
                                  in_=xT_d.ap()[:, :, csl])

            nc.sync.dma_start(out=wqT_sb, in_=wqT_d.ap())
            load_x_cb(0)
            nc.sync.dma_start(out=wkT_sb, in_=wkT_d.ap())
            load_x_cb(1)
            nc.sync.dma_start(out=wvT_sb, in_=wvT_d.ap())
            load_x_cb(2)
            load_x_cb(3)

            woT_sb = const.tile([128, 2, C], BF16)
            for p in range(2):
                nc.sync.dma_start(out=woT_sb[:, p, :], in_=woT_d.ap()[p])
            tdiag_sb = const.tile([128, 4, JT], BF16)
            for s in range(4):
                nc.sync.dma_start(out=tdiag_sb[:, s, :], in_=tdiag_d.ap()[s])
            ident_sb = const.tile([128, JT], BF16)
            nc.sync.dma_start(out=ident_sb, in_=ident_d.ap())

            qaug_sb = []
            khl_sb = []
            khu_sb = []
            for s in range(4):
                qa = const.tile([AUGP, L], BF16, tag=f"qaug{s}", name=f"qaug{s}")
                kl = const.tile([AUGP, L], BF16, tag=f"khl{s}", name=f"khl{s}")
                ku = const.tile([AUGP, L], BF16, tag=f"khu{s}", name=f"khu{s}")
                nc.sync.dma_start(out=qa[D:AUGP, :], in_=augq_d.ap()[s])
                nc.sync.dma_start(out=kl[D:AUGP, :], in_=augkl_d.ap()[s])
                nc.sync.dma_start(out=ku[D:AUGP, :], in_=augku_d.ap()[s])
                qaug_sb.append(qa)
                khl_sb.append(kl)
                khu_sb.append(ku)

            # V with ones column appended (PV ones-row => softmax denominator)
            vh_sb = const.tile([128, L // JT, 4, D + 1], BF16)
            nc.vector.memset(vh_sb[:, :, :, D:D + 1], 1.0)

            # rank-1 zero operand: zeroes a pv tile in one 512-moving matmul
            zrow_sb = const.tile([1, IT], BF16, tag="zrow", name="zrow")
            nc.vector.memset(zrow_sb, 0.0)

            outT_sb = [const.tile([128, L], BF16, tag=f"outT{p}",
                                  name=f"outT{p}") for p in range(2)]

            # ---- q/k projections (lp-outer: earliest columns first) ----
            for lp in range(L // PAIR):
                for ct in range(2):
                    psq = psm.tile([128, PAIR], F32, tag="ps")
                    psk = psm.tile([128, PAIR], F32, tag="ps")
                    lsl = slice(lp * PAIR, (lp + 1) * PAIR)
                    for hf in range(2):
                        hsl = slice(lp * PAIR + hf * IT,
                                    lp * PAIR + (hf + 1) * IT)
                        for kt in range(KT):
                            nc.tensor.matmul(
                                psq[:, hf * IT:(hf + 1) * IT],
                                lhsT=wqT_sb[:, kt, ct * 128:(ct + 1) * 128],
                                rhs=xT_sb[:, kt, hsl],
                                start=(kt == 0), stop=(kt == KT - 1))
                    for hf in range(2):
                        hsl = slice(lp * PAIR + hf * IT,
                                    lp * PAIR + (hf + 1) * IT)
                        for kt in range(KT):
                            nc.tensor.matmul(
                                psk[:, hf * IT:(hf + 1) * IT],
                                lhsT=wkT_sb[:, kt, ct * 128:(ct + 1) * 128],
                                rhs=xT_sb[:, kt, hsl],
                                start=(kt == 0), stop=(kt == KT - 1))
                    for half in range(2):
                        s = ct * 2 + half
                        sl = slice(half * D, (half + 1) * D)
                        nc.vector.tensor_copy(out=qaug_sb[s][0:D, lsl],
                                              in_=psq[sl, :])
                        # k casts ride the idle scalar engine
                        nc.scalar.copy(out=khl_sb[s][0:D, lsl],
                                       in_=psk[sl, :])
                        # second copy of k base rows via DMA (off-engine)
                        nc.sync.dma_start(out=khu_sb[s][0:D, lsl],
                                          in_=khl_sb[s][0:D, lsl])
            # ---- v projection, [L, channel] layout ----
            for jt in range(L // JT):
                psv = psy.tile([128, 4 * D], F32, tag="pso")
                for kt in range(KT):
                    nc.tensor.matmul(
                        psv, lhsT=xT_sb[:, kt, jt * JT:(jt + 1) * JT],
                        rhs=wvT_sb[:, kt, :],
                        start=(kt == 0), stop=(kt == KT - 1))
                for s in range(4):
                    if s % 2 == 0:
                        nc.vector.tensor_copy(
                            out=vh_sb[:, jt, s, 0:D],
                            in_=psv[:, s * D:(s + 1) * D])
                    else:
                        nc.scalar.copy(
                            out=vh_sb[:, jt, s, 0:D],
                            in_=psv[:, s * D:(s + 1) * D])

            # ---- attention (banded), 2-deep software pipeline ----
            # Scores+exp for tile i+2 are emitted before the PV matmuls of
            # tile i, across slot boundaries, so the PE never waits for the
            # exp at a chain start (psm has exactly 2 score buffers).
            den_all = {}
            u_map = {}
            dstage_map = {}
            pv_state = {}

            def make_tiles(m, s):
                I_a, I_b = 2 * m, 2 * m + 1
                wins_a = {j: w for j in range(L // JT)
                          if (w := _win(s, j, I_a))}
                wins_b = {j: w for j in range(L // JT)
                          if (w := _win(s, j, I_b))}
                js_a = sorted(wins_a)
                js_b = sorted(wins_b)
                js_all = sorted(set(js_a) | set(js_b))
                info = {}
                for I, js, wins in ((I_a, js_a, wins_a), (I_b, js_b, wins_b)):
                    last_j = {}
                    for j in js:
                        for c in range(*wins[j]):
                            last_j[c] = j
                    dead = [c for c in range(IT // JT) if c not in last_j]
                    info[I] = dict(js=js, wins=wins, last_j=last_j, dead=dead)
                return [dict(m=m, s=s, j=j, js_a=js_a, js_b=js_b, info=info)
                        for j in js_all]

            def emit_score(t):
                m, s, j = t['m'], t['s'], t['j']
                base_i = m * PAIR
                in_a, in_b = j in t['js_a'], j in t['js_b']
                # union window over the pair, in 128-chunk units (0..8):
                # halves' windows are edge-aligned so the union is
                # contiguous
                wa = t['info'][2 * m]['wins'].get(j)
                wb = t['info'][2 * m + 1]['wins'].get(j)
                u0 = wa[0] if in_a else 4 + wb[0]
                u1 = 4 + wb[1] if in_b else wa[1]
                cl, ch = u0 * JT, u1 * JT
                st = psm.tile([128, PAIR], F32, tag="ps")
                J0 = j * JT
                if base_i <= J0 < base_i + PAIR:
                    # diagonal pair: runs either side of the diagonal
                    # 128-chunk, split into <=512-wide matmuls
                    cd = (J0 - base_i) // JT
                    c_lo, c_hi = u0, u1

                    def run_mm(lo, hi, kh):
                        # merge chunks, splitting at PSUM bank edges
                        # (a matmul output must stay in one 2KB bank)
                        while lo < hi:
                            bnd = (lo // 4 + 1) * 4
                            n = min(hi, bnd) - lo
                            nc.tensor.matmul(
                                st[:, lo * JT:(lo + n) * JT],
                                lhsT=kh[:, slice(J0, J0 + JT)],
                                rhs=qaug_sb[s][:, base_i + lo * JT:
                                               base_i + (lo + n) * JT],
                                start=True, stop=True)
                            lo += n

                    run_mm(c_lo, cd, khu_sb[s])
                    # bias then scores, accumulated on the PE: the
                    # identity rhs materializes tdiag^T (symmetric)
                    nc.tensor.matmul(
                        st[:, cd * JT:(cd + 1) * JT],
                        lhsT=tdiag_sb[:, s, :], rhs=ident_sb,
                        start=True, stop=False)
                    nc.tensor.matmul(
                        st[:, cd * JT:(cd + 1) * JT],
                        lhsT=khl_sb[s][0:D, J0:J0 + JT],
                        rhs=qaug_sb[s][0:D, J0:J0 + JT],
                        start=False, stop=True)
                    run_mm(cd + 1, c_hi, khl_sb[s])
                else:
                    kh = khl_sb[s] if J0 < base_i else khu_sb[s]
                    lo = u0
                    while lo < u1:   # split at the PSUM bank edge (4)
                        hi = min(u1, (lo // 4 + 1) * 4)
                        nc.tensor.matmul(
                            st[:, lo * JT:hi * JT],
                            lhsT=kh[:, J0:J0 + JT],
                            rhs=qaug_sb[s][:, base_i + lo * JT:
                                           base_i + hi * JT],
                            start=True, stop=True)
                        lo = hi
                p = ppool.tile([128, PAIR], BF16, tag="p")
                nc.scalar.activation(out=p[:, cl:ch], in_=st[:, cl:ch],
                                     func=mybir.ActivationFunctionType.Exp)
                t['p'] = p

            def emit_pv(t):
                m, s, j = t['m'], t['s'], t['j']
                pv = pv_state.setdefault((m, s), {})
                for I in (2 * m, 2 * m + 1):
                    info = t['info'][I]
                    js = info['js']
                    if j not in js:
                        continue
                    hoff = (I % 2) * IT
                    w0, w1 = info['wins'][j]
                    if j == js[0]:
                        pv[I] = pspv.tile([D + 1, IT], F32, tag="pv",
                                          name=f"pv{s}_{I}")
                        # first write starts its own window; the rest of
                        # the tile is zeroed by rank-1 matmuls (per-chunk
                        # start flags would interleave accumulation
                        # groups in one bank, which misbehaves on HW)
                        for z0, z1 in ((0, w0), (w1, IT // JT)):
                            if z0 < z1:
                                nc.tensor.matmul(
                                    pv[I][:, z0 * JT:z1 * JT],
                                    lhsT=zrow_sb[:, 0:D + 1],
                                    rhs=zrow_sb[:, 0:(z1 - z0) * JT],
                                    start=True, stop=True)
                    nc.tensor.matmul(
                        pv[I][:, w0 * JT:w1 * JT],
                        lhsT=vh_sb[:, j, s, :],
                        rhs=t['p'][:, hoff + w0 * JT:hoff + w1 * JT],
                        start=(j == js[0]), stop=(j == js[-1]),
                        skip_group_check=True)
                    if j == js[-1]:
                        # chain done: stage unnormalized out + den row,
                        # freeing the pv bank promptly. The den row is
                        # DMA-reshaped onto 16 partitions so the batched
                        # reciprocal runs 128-wide. For the pair's last
                        # slot the den copy goes first: it gates the
                        # reciprocal -> oproj critical path. Chunks no
                        # kept key tile wrote are zero from the pv init:
                        # set den to 1 there (recip(0) is NaN and would
                        # poison the 0*rec multiply).
                        u = upool.tile([D, IT], F32, tag="u",
                                       name=f"u{s}_{I}")
                        r = s * 2 + (I % 2)
                        dstage = dnb.tile([1, IT], F32, tag="dstage",
                                          name=f"dstage{s}_{I}")

                        def stage_den(pvI=pv[I], dstage=dstage,
                                      dead=info['dead']):
                            nc.vector.tensor_copy(out=dstage,
                                                  in_=pvI[D:D + 1, :])
                            for c in dead:
                                nc.vector.memset(
                                    dstage[:, c * JT:(c + 1) * JT], 1.0)

                        def stage_u(pvI=pv[I], u=u):
                            nc.vector.tensor_copy(out=u, in_=pvI[0:D, :])

                        if s == 3:
                            stage_den()
                            stage_u()
                        else:
                            stage_u()
                            stage_den()
                        nc.sync.dma_start(
                            out=den_all[m][r * 16:(r + 1) * 16, :],
                            in_=dstage)
                        u_map[(s, I)] = u
                        dstage_map[(s, I)] = dstage

            def norm_batch(m, srange, tagb, halves=(0, 1), split=False):
                # batched 128-wide reciprocal of the den rows present so
                # far (row r on partitions 16r..16r+15; unwritten rows
                # produce garbage that is never broadcast), bf16 bounce
                # through DRAM (linear layout matches the [8, IT] row
                # view), then per-row broadcast + multiply into outT.
                # I_a rows first so oproj's first lt chunks unblock early.
                recf = dnb.tile([128, IT // 16], F32, tag="recf",
                                name=f"recf{m}{tagb}")
                nc.vector.reciprocal_approx_fast(out=recf, in_=den_all[m])
                denb = dnb.tile([128, IT // 16], BF16, tag="dnb",
                                name=f"dnb{m}{tagb}")
                nc.vector.tensor_copy(out=denb, in_=recf)
                dbounce = dpool.tile([8, IT], BF16, tag="dbounce",
                                     name=f"dbounce{m}{tagb}")
                nc.sync.dma_start(out=dbounce, in_=denb)
                for half in halves:
                    I = 2 * m + half
                    for s in srange:
                        r = s * 2 + half
                        rec = recpool.tile([D, IT], BF16, tag="rec",
                                           name=f"rec{s}_{I}")
                        # broadcasts issue off the busy sync queue; on the
                        # critical tail they split across two queues to
                        # halve the single-engine 64KB flight time
                        src = dbounce[r:r + 1, :]
                        if split:
                            nc.gpsimd.dma_start(
                                out=rec[0:D // 2, :],
                                in_=src.to_broadcast([D // 2, IT]))
                            nc.sync.dma_start(
                                out=rec[D // 2:D, :],
                                in_=src.to_broadcast([D // 2, IT]))
                        else:
                            eng = nc.gpsimd if half == 0 else nc.sync
                            eng.dma_start(out=rec,
                                          in_=src.to_broadcast([D, IT]))
                        dst = slice((s % 2) * D, (s % 2) * D + D)
                        nc.vector.tensor_mul(
                            out=outT_sb[s // 2][dst, I * IT:(I + 1) * IT],
                            in0=u_map[(s, I)], in1=rec)

            y0s = {}

            def oproj_passA(m):
                # last pair: the slot-0/1 contraction half only needs the
                # early-normalized outT rows, so its matmuls fill the PE
                # while slot 3's normalization lands; copies ride the ACT
                # engine (the DVE queue is parked on DMA-gated mults).
                for lt in range(m * PAIR // JT, (m + 1) * PAIR // JT):
                    ls = slice(lt * JT, (lt + 1) * JT)
                    for hf in range(2):
                        hsl = slice(hf * IT, (hf + 1) * IT)
                        pso = psy.tile([128, IT], F32, tag="pso")
                        nc.tensor.matmul(pso, lhsT=outT_sb[0][:, ls],
                                         rhs=woT_sb[:, 0, hsl],
                                         start=True, stop=True)
                        y0 = ypool.tile([128, IT], BF16, tag="y0")
                        nc.scalar.copy(out=y0, in_=pso)
                        y0s[(lt, hf)] = y0

            def oproj(m, split_acc=False):
                lts = range(m * PAIR // JT, (m + 1) * PAIR // JT)
                for lt in lts:
                    ls = slice(lt * JT, (lt + 1) * JT)
                    for hf in range(2):
                        hsl = slice(hf * IT, (hf + 1) * IT)
                        pso = psy.tile([128, IT], F32, tag="pso")
                        if not split_acc:
                            nc.tensor.matmul(pso, lhsT=outT_sb[0][:, ls],
                                             rhs=woT_sb[:, 0, hsl],
                                             start=True, stop=False)
                        nc.tensor.matmul(pso, lhsT=outT_sb[1][:, ls],
                                         rhs=woT_sb[:, 1, hsl],
                                         start=split_acc, stop=True)
                        y = ypool.tile([128, IT], BF16, tag="y")
                        if split_acc:
                            nc.vector.tensor_add(y, y0s[(lt, hf)], pso)
                        elif (lt + hf) % 2 == 0:
                            nc.vector.tensor_copy(out=y, in_=pso)
                        else:
                            nc.scalar.copy(out=y, in_=pso)
                        # alternate issue queues: 16 back-to-back issues on
                        # one sequencer would clog it right when the next
                        # norm batch needs it
                        eng = nc.gpsimd if (lt + hf) % 2 == 0 else nc.sync
                        eng.dma_start(out=out_d.ap()[ls, hsl], in_=y)

            for m in range(L // PAIR):
                den_all[m] = rpool.tile([128, IT // 16], F32, tag="den",
                                        name=f"den{m}")

            # slots 0-2's den rows normalize early (hidden under slot 3);
            # only slot 3's two rows sit on the tail critical path, and
            # its I_a row is processed the moment that chain ends.
            # oproj(0) lands after attn(1,0) so its normalization DMAs are
            # covered by PE work; the pipeline keeps emitting scores of
            # the next slot ahead of the hooks.
            slots = [(m, s) for m in range(L // PAIR) for s in range(4)]
            hooks = {
                (0, 2, 'end'): [lambda: norm_batch(0, range(3), "a")],
                (0, 3, 'a'): [lambda: norm_batch(0, [3], "b", halves=(0,),
                                                 split=True)],
                (0, 3, 'end'): [lambda: norm_batch(0, [3], "c", halves=(1,),
                                                   split=True)],
                (1, 0, 'end'): [lambda: oproj(0)],
                (1, 2, 'end'): [lambda: norm_batch(1, range(3), "a")],
                (1, 3, 'a'): [lambda: norm_batch(1, [3], "b", halves=(0,),
                                                 split=True)],
                (1, 3, 'end'): [lambda: oproj_passA(1),
                                lambda: norm_batch(1, [3], "c", halves=(1,),
                                                   split=True),
                                lambda: oproj(1, split_acc=True)],
            }
            DEPTH = 2
            tiles = []
            hook_at = {}
            for (m, s) in slots:
                ts = make_tiles(m, s)
                js_a = ts[0]['js_a']
                base = len(tiles)
                tiles += ts
                ia = next(i for i, t in enumerate(ts)
                          if js_a and t['j'] == js_a[-1])
                if (m, s, 'a') in hooks:
                    hook_at.setdefault(base + ia, []).extend(
                        hooks[(m, s, 'a')])
                if (m, s, 'a2') in hooks:
                    idx = base + max(ia, len(ts) - 1 - DEPTH)
                    hook_at.setdefault(idx, []).extend(hooks[(m, s, 'a2')])
                if (m, s, 'end') in hooks:
                    hook_at.setdefault(len(tiles) - 1, []).extend(
                        hooks[(m, s, 'end')])

            def retire(i):
                emit_pv(tiles[i])
                for fn in hook_at.get(i, ()):
                    fn()

            for i, t in enumerate(tiles):
                emit_score(t)
                if i >= DEPTH:
                    retire(i - DEPTH)
            for i in range(len(tiles) - DEPTH, len(tiles)):
                retire(i)

    nc.compile()
    return nc


def _prep_core_inputs(q, Wq, Wk, Wv, Wo, b, g):
    heads = GROUPS[g]
    KT = C // 128

    def chunked(a):  # [C, n] -> [128, KT, n] (contraction pre-chunked)
        n = a.shape[1]
        return np.ascontiguousarray(
            a.reshape(KT, 128, n).transpose(1, 0, 2)).astype(BF16_NP)

    xT = chunked(q[b].T)

    def stackT(W, scale=1.0):
        rows = np.concatenate([W[h * D:(h + 1) * D, :] for h in heads], axis=0)
        return chunked(rows.T * scale)

    wqT = stackT(Wq, 1.0 / np.sqrt(D))
    wkT = stackT(Wk)
    wvT = stackT(Wv)
    woT = np.stack([
        np.concatenate([np.ascontiguousarray(Wo[:, h * D:(h + 1) * D].T)
                        for h in heads[2 * p:2 * p + 2]], axis=0)
        for p in range(2)]).astype(BF16_NP)

    jj = np.arange(L)
    bits = ((jj[None, :] >> np.arange(NB)[:, None]) & 1).astype(np.float32)
    ones = np.ones((NB, L), dtype=np.float32)
    augq = np.zeros((4, 2 * NB, L), dtype=np.float32)
    augkl = np.zeros((4, 2 * NB, L), dtype=np.float32)
    augku = np.zeros((4, 2 * NB, L), dtype=np.float32)
    tdiag = np.zeros((4, JT, JT), dtype=np.float32)
    for s, h in enumerate(heads):
        sb = float(np.float32(SLOPES[h]).astype(BF16_NP).astype(np.float32))
        planes = (bits * (2.0 ** np.arange(NB))[:, None] * sb).astype(BF16_NP)
        planes = planes.astype(np.float32)  # exact bf16 values
        augq[s] = np.concatenate([ones, planes], axis=0)
        augkl[s] = np.concatenate([planes, -ones], axis=0)
        augku[s] = np.concatenate([-planes, ones], axis=0)
        d = np.abs(jj[:JT][None, :] - jj[:JT][:, None]).astype(np.float32)
        tdiag[s] = -sb * d
    return {
        "xT": xT, "wqT": wqT, "wkT": wkT, "wvT": wvT, "woT": woT,
        "augq": augq.astype(BF16_NP), "augkl": augkl.astype(BF16_NP),
        "augku": augku.astype(BF16_NP), "tdiag": tdiag.astype(BF16_NP),
        "ident": np.eye(JT, dtype=BF16_NP),
    }


def kernel(q, Wq, Wk, Wv, Wo, bo):
    global _last_results
    q = np.asarray(q, dtype=np.float32)
    Wq = np.asarray(Wq, dtype=np.float32)
    Wk = np.asarray(Wk, dtype=np.float32)
    Wv = np.asarray(Wv, dtype=np.float32)
    Wo = np.asarray(Wo, dtype=np.float32)
    bo = np.asarray(bo, dtype=np.float32)

    trace = bool(os.environ.get("BASS_TRACE"))
    if trace:
        _install_axon_prof_shim()
    from concourse.bass_utils import run_bass_kernel_spmd

    nc = build_bass()
    in_maps = [_prep_core_inputs(q, Wq, Wk, Wv, Wo, core // 4, core % 4)
               for core in range(N_CORES)]
    res = run_bass_kernel_spmd(nc, in_maps, core_ids=list(range(N_CORES)),
                               trace=trace)
    _last_results = res
    B = q.shape[0]
    out = np.zeros((B, L, C), dtype=np.float32)
    for core in range(N_CORES):
        out[core // 4] += res.results[core]["out"].astype(np.float32)
    out += bo[None, None, :]
    return out


def _install_axon_prof_shim():
    """Provide the missing antenv.axon_hooks so trace=True works under axon."""
    import contextlib
    import ctypes
    import sys
    import types

    if "antenv.axon_hooks" in sys.modules:
        return
    so_path = "/opt/axon/libaxon_pjrt.so"
    try:
        lib = ctypes.CDLL(so_path)
    except OSError:
        return
    if not hasattr(lib, "axon_start_nrt_profile"):
        return
    lib.axon_start_nrt_profile.argtypes = [ctypes.POINTER(ctypes.c_int64),
                                           ctypes.c_size_t]
    lib.axon_start_nrt_profile.restype = ctypes.c_int64
    lib.axon_stop_nrt_profile.argtypes = [ctypes.c_char_p]
    lib.axon_stop_nrt_profile.restype = ctypes.c_int64

    @contextlib.contextmanager
    def _hook(output_dir, device_ids):
        import jax
        jax.devices()
        if device_ids:
            ids = (ctypes.c_int64 * len(device_ids))(*device_ids)
            rc = lib.axon_start_nrt_profile(ids, len(device_ids))
        else:
            rc = lib.axon_start_nrt_profile(None, 0)
        if rc != 0:
            raise RuntimeError(f"axon_start_nrt_profile rc={rc}")
        try:
            yield
        finally:
            n = lib.axon_stop_nrt_profile(str(output_dir).encode())
            print(f"profile: {n} file(s) -> {output_dir}", file=sys.stderr)

    mod = types.ModuleType("antenv.axon_hooks")
    mod.get_axon_ntff_profile_hook = lambda: _hook
    mod.set_axon_ntff_profile_hook = lambda h: None
    sys.modules["antenv.axon_hooks"] = mod
    try:
        import antenv
        antenv.axon_hooks = mod
    except ImportError:
        pass
